# revision 12
# baseline (speedup 1.0000x reference)
"""MHA forward kernel for Trainium2 (Bass/Tile), sharded over (batch, head)
pairs across 8 NeuronCores.

Math (per (b,h) pair):
    out = softmax(Q K^T / sqrt(64) + bias) @ V     # bias broadcast over (b,h)

Device-side decomposition (everything transposed so the S x S score matrix
never needs an on-chip transpose):
    scoresT[k, q] = sum_d K[k,d] Q'[q,d]          (Q' = Q/8, host pre-scaled)
    es = exp(scoresT)                              (ACT, PSUM -> SBUF bf16)
    p  = es * ebiasT                               (DVE 2x mode, ebiasT =
                                                    exp(bias)^T host-precomputed
                                                    bf16; exp(s+b)=exp(s)exp(b))
    outT[d, q], sums[q] = [V | ones] matmul accum over k   (PE)
    host epilogue: out[q, d] = outT[d, q] / sums[q]

Schedule: qt-major over pairs — for each q-tile, all 4 pairs run before
moving to the next q-tile, so each exp(bias) q-chunk (2.1 MB) covers
~35us of compute (~60 GB/s sustained vs 240 GB/s if pair-major). ACT is
the pacing engine (~1.09us per 2-ktile group); PE (MM1+MM2) hides under
it. Final divide + transpose runs on host over the gathered f32 output.
"""

import os
import sys

import numpy as np

for _p in ("/opt/trn_rl_repo",):
    if _p not in sys.path and os.path.isdir(_p):
        sys.path.insert(0, _p)

B, H, S, D = 2, 16, 2048, 64
N_CORES = 8
PAIRS = B * H                     # 32
PPC = PAIRS // N_CORES            # 4 pairs per core
SCALE = 1.0 / 8.0                 # 1/sqrt(64)

KT = S // 128                     # 16 k-tiles of 128
QTILE = 512
QT = S // QTILE                   # 4 q-tiles
# k-tile chunking per (pair, qt): 6 chunks instead of 8 cuts the ~239ns
# fixed cost per ACT exp instruction; 3-ktile chunks are 3 PSUM banks, so
# sc double-buffers at 2x3 banks + 2 accumulator banks = 8 exactly.
CHUNKS = [(0, 3), (3, 6), (6, 9), (9, 12), (12, 14), (14, 16)]
NG = len(CHUNKS)
SC_BUFS = int(os.environ.get("SC_BUFS", "2"))
ES_BUFS = int(os.environ.get("ES_BUFS", "3"))
P_BUFS = int(os.environ.get("P_BUFS", "3"))
LAG = int(os.environ.get("LAG", "2"))

_CACHE = {}


def _build_nc():
    import concourse.mybir as mybir
    import concourse.tile as tile
    from concourse import bacc

    f32 = mybir.dt.float32
    bf16 = mybir.dt.bfloat16
    nc = bacc.Bacc(None)

    qT = nc.declare_dram_parameter("qT", [PPC, D, S], bf16, isOutput=False)
    kT = nc.declare_dram_parameter("kT", [PPC, D, S], bf16, isOutput=False)
    # v1: [pair][p][kt][d] so each partition's line is 16*65*2B contiguous
    v1 = nc.declare_dram_parameter("v1", [PPC, 128, KT, D + 1], bf16, isOutput=False)
    # ebT: [qt][p][kt][q] so a qt-chunk DMA is 16 KiB contiguous per partition
    ebT = nc.declare_dram_parameter("ebT", [QT, 128, KT, QTILE], bf16, isOutput=False)
    outT = nc.declare_dram_parameter("outT", [PPC, QT, D + 1, QTILE], f32, isOutput=True)

    with tile.TileContext(nc) as tc:
        with (
            tc.tile_pool(name="eb", bufs=1) as eb_pool,
            tc.tile_pool(name="qk", bufs=1) as qk_pool,
            tc.tile_pool(name="vv", bufs=1) as v_pool,
            tc.tile_pool(name="es", bufs=ES_BUFS) as es_pool,
            tc.tile_pool(name="pp", bufs=P_BUFS) as p_pool,
            tc.tile_pool(name="ob", bufs=2) as ob_pool,
            tc.tile_pool(name="sc", bufs=SC_BUFS, space="PSUM") as sc_pool,
            tc.tile_pool(name="acc", bufs=2, space="PSUM") as acc_pool,
        ):
            # SBUF layout [p, qt, kt, q]: a qt-chunk is contiguous per
            # partition on both sides -> few, large DMA descriptors.
            eb_sb = eb_pool.tile([128, QT, KT, QTILE], bf16)

            # Pair 0 chunked small so MM1 starts within ~1us of launch; the
            # ebT qt0 stream follows immediately (needed from the first mult).
            qk_tiles = {}
            for p in range(PPC):
                q_sb = qk_pool.tile([D, S], bf16, name="q_sb", tag=f"q{p}")
                k_sb = qk_pool.tile([D, S], bf16, name="k_sb", tag=f"k{p}")
                qk_tiles[p] = (q_sb, k_sb)
                if p == 0:
                    # fine-grained first loads: kT group-0 columns + qT qt0
                    nc.sync.dma_start(k_sb[:, 0:256], kT[p][:, 0:256])
                    nc.sync.dma_start(q_sb[:, 0:QTILE], qT[p][:, 0:QTILE])
                    nc.sync.dma_start(k_sb[:, 256:1024], kT[p][:, 256:1024])
                    nc.sync.dma_start(k_sb[:, 1024:2048], kT[p][:, 1024:2048])
                    nc.sync.dma_start(q_sb[:, QTILE:], qT[p][:, QTILE:])

            v_tiles = {}
            v_tiles[0] = v_pool.tile([128, KT, D + 1], bf16, name="v_sb", tag="v0")
            nc.sync.dma_start(v_tiles[0][:], v1[0])
            # qt0 exp(bias) in 4-ktile sub-chunks so the first mults unblock
            # within a few us; later q-tiles stream as whole 2.1 MB chunks.
            for kc in range(0, KT, 4):
                nc.sync.dma_start(
                    eb_sb[:, 0, kc : kc + 4, :], ebT[0][:, kc : kc + 4, :]
                )
            for p in range(1, PPC):
                q_sb, k_sb = qk_tiles[p]
                nc.sync.dma_start(q_sb[:], qT[p])
                nc.sync.dma_start(k_sb[:], kT[p])
                v_tiles[p] = v_pool.tile(
                    [128, KT, D + 1], bf16, name="v_sb", tag=f"v{p}"
                )
                nc.sync.dma_start(v_tiles[p][:], v1[p])
            for qc in range(1, QT):
                nc.sync.dma_start(eb_sb[:, qc], ebT[qc])

            # ---- chunk stream: qt-major over pairs --------------------------
            stream = []  # (p, qt, g)
            for qt in range(QT):
                for p in range(PPC):
                    for g in range(NG):
                        stream.append((p, qt, g))

            state = {}

            def produce(p, qt, g):
                kt0, kt1 = CHUNKS[g]
                nk = kt1 - kt0
                q_sb, k_sb = qk_tiles[p]
                qs = q_sb[:, qt * QTILE : (qt + 1) * QTILE]
                s_psum = sc_pool.tile([128, 3, QTILE], f32, tag="sc")
                for j in range(nk):
                    kt = kt0 + j
                    nc.tensor.matmul(
                        s_psum[:, j, :],
                        k_sb[:, kt * 128 : (kt + 1) * 128],
                        qs,
                        start=True,
                        stop=True,
                    )
                es = es_pool.tile([128, 3, QTILE], bf16, tag="es")
                nc.scalar.activation(
                    es[:, :nk, :], s_psum[:, :nk, :],
                    mybir.ActivationFunctionType.Exp,
                )
                p_sb = p_pool.tile([128, 3, QTILE], bf16, tag="p")
                nc.vector.tensor_mul(
                    p_sb[:, :nk, :],
                    es[:, :nk, :],
                    eb_sb[:, qt, kt0:kt1, :],
                )
                return p_sb

            def consume(p, qt, g, p_sb):
                kt0, kt1 = CHUNKS[g]
                v_sb = v_tiles[p]
                st = state[(p, qt)]
                for j in range(kt1 - kt0):
                    kt = kt0 + j
                    nc.tensor.matmul(
                        st,
                        v_sb[:, kt, :],
                        p_sb[:, j, :],
                        start=(kt == 0),
                        stop=(kt == KT - 1),
                    )

            def epilogue(p, qt):
                o_psum = state.pop((p, qt))
                o_sb = ob_pool.tile([D + 1, QTILE], f32, tag="osb")
                nc.vector.tensor_scalar_mul(o_sb[:], o_psum[:], 1.0)
                nc.sync.dma_start(outT[p, qt], o_sb[:])

            pending = []  # (p, qt, g, p_sb)
            for p, qt, g in stream:
                if (p, qt) not in state:
                    state[(p, qt)] = acc_pool.tile(
                        [D + 1, QTILE], mybir.dt.float32, name="osum", tag="osum"
                    )
                p_sb = produce(p, qt, g)
                if len(pending) >= LAG:
                    pp, pq, pg, ps = pending.pop(0)
                    consume(pp, pq, pg, ps)
                    if pg == NG - 1:
                        epilogue(pp, pq)
                pending.append((p, qt, g, p_sb))
            while pending:
                pp, pq, pg, ps = pending.pop(0)
                consume(pp, pq, pg, ps)
                if pg == NG - 1:
                    epilogue(pp, pq)

    return nc


def _get_nc():
    if "nc" not in _CACHE:
        nc = _build_nc()
        nc.finalize()
        _CACHE["nc"] = nc
    return _CACHE["nc"]


def _make_in_maps(mat1, mat2, mat3, bias):
    import ml_dtypes

    bf16 = ml_dtypes.bfloat16
    q = np.asarray(mat1, dtype=np.float32).reshape(PAIRS, S, D) * np.float32(SCALE)
    k = np.asarray(mat2, dtype=np.float32).reshape(PAIRS, S, D)
    v = np.asarray(mat3, dtype=np.float32).reshape(PAIRS, S, D)
    v1 = np.concatenate([v, np.ones((PAIRS, S, 1), np.float32)], axis=2)
    # [pair][kt*128+p][d] -> [pair][p][kt][d]
    v1 = np.ascontiguousarray(
        v1.reshape(PAIRS, KT, 128, D + 1).transpose(0, 2, 1, 3).astype(bf16)
    )
    ebt = np.exp(np.asarray(bias, dtype=np.float32).reshape(S, S)).T  # [k, q]
    # [kt*128+p][qt*512+q] -> [qt][p][kt][q]
    ebT = np.ascontiguousarray(
        ebt.reshape(KT, 128, QT, QTILE).transpose(2, 1, 0, 3).astype(bf16)
    )

    in_maps = []
    for c in range(N_CORES):
        sl = slice(c * PPC, (c + 1) * PPC)
        in_maps.append(
            {
                "qT": np.ascontiguousarray(q[sl].transpose(0, 2, 1).astype(bf16)),
                "kT": np.ascontiguousarray(k[sl].transpose(0, 2, 1).astype(bf16)),
                "v1": v1[sl],
                "ebT": ebT,
            }
        )
    return in_maps


def kernel(mat1, mat2, mat3, bias):
    from concourse.bass_utils import run_bass_kernel_spmd

    in_maps = _make_in_maps(mat1, mat2, mat3, bias)
    nc = _get_nc()
    _CACHE["in_maps"] = in_maps
    res = run_bass_kernel_spmd(nc, in_maps, list(range(N_CORES)))
    outs = []
    for c in range(N_CORES):
        oT = res.results[c]["outT"]            # [PPC, QT, D+1, QTILE] f32
        oT = oT.transpose(0, 2, 1, 3).reshape(PPC, D + 1, S)
        o = oT[:, :D, :] / oT[:, D : D + 1, :]  # divide by softmax sums
        outs.append(o.transpose(0, 2, 1))       # [PPC, S, D]
    full = np.concatenate(outs, axis=0).reshape(B, H, S, D)
    return np.ascontiguousarray(full.astype(np.float32))
